# revision 15
# baseline (speedup 1.0000x reference)
"""GCNConv kernel for 8x Trainium2 NeuronCores (Bass/Tile).

Reference computation:
    h = x @ W + b                  # [N, 256] @ [256, 128] -> [N, 128]
    out[i] = sum_{e: dst[e]=i} val[e] * h[src[e]]

Strategy (per core; SPMD - one program, per-core data):
  - dst nodes sharded 12500/core (output rows).  Edges partitioned by dst.
  - Phase 1: every core computes the full h (fp16) into 4 per-window DRAM
    tensors via PE matmuls (host passes x transposed + fp16).  Per-window
    tensors let window-0 gathers start as soon as 1/4 of h is written.
  - Phase 2: edges sorted by (win, grp, tile, dst); per (win, grp) bucket
    one dma_gather pulls the per-edge h rows on-chip (int16 indices within
    the 25088-row window).  Slot padding uses trailing -1 indices, which
    the Q7 descriptor generator truncates for free.  Per 128-edge chunk a
    host-built staircase matrix B [128e x 32seg] fp16 (carrying val) is the
    stationary matmul operand -> PSUM partial segment sums; a second
    one-hot S2 [128seg x 128dst] fp16 matmul accumulates segments into
    per-dst-tile PSUM, which is added into an SBUF fp32 accumulator.
  - Bias is folded in at eviction: out_tile = acc + deg (x) bias.
All data-dependent structure is padded to the max across cores so the same
program serves all 8 cores; pad slots carry idx=-1 and zero B columns.
"""

import sys

for _p in ("/opt/trn_rl_repo",):
    if _p not in sys.path:
        sys.path.insert(0, _p)

import numpy as np

P = 128
MSEG = 32            # segment slots per 128-edge chunk
TPG = 4              # dst tiles (of 128 dst) per processing group
N_CORES = 8
NWIN = 4
WINR = 25088         # rows per gather window (<= 32767 for int16 idx)


def _ceil_to(a, m):
    return -(-a // m) * m


class Plan:
    """Static (core-invariant) program structure + per-core data arrays."""


def build_plan(x, edge_src, edge_dst, edge_vals, weight, bias):
    N, IN_F = x.shape
    OUT_F = weight.shape[1]
    assert N % N_CORES == 0
    ndst = N // N_CORES                    # dst nodes per core
    ndst_pad = _ceil_to(ndst, P)
    ntile = ndst_pad // P                  # dst tiles per core
    ngrp = -(-ntile // TPG)
    hrows = _ceil_to(N, P)
    assert (NWIN - 1) * WINR < hrows <= NWIN * WINR
    win_rows = [min(WINR, hrows - w * WINR) for w in range(NWIN)]

    pl = Plan()
    pl.N, pl.IN_F, pl.OUT_F = N, IN_F, OUT_F
    pl.ndst, pl.ndst_pad, pl.ntile, pl.ngrp = ndst, ndst_pad, ntile, ngrp
    pl.hrows, pl.win_rows = hrows, win_rows
    pl.kc = IN_F // P                      # K chunks for projection

    # --- dense inputs ---
    xT = np.zeros((pl.kc, P, hrows), np.float16)
    xT[:, :, :N] = np.ascontiguousarray(x.astype(np.float16).T).reshape(
        pl.kc, P, N
    )
    pl.xT = xT
    pl.W = np.ascontiguousarray(
        weight.astype(np.float16).reshape(pl.kc, P, OUT_F).transpose(1, 0, 2)
    )  # [P, kc, OUT_F]
    pl.b128 = np.ascontiguousarray(
        np.broadcast_to(bias.astype(np.float32)[None, :], (P, OUT_F))
    )  # [P, OUT_F] bias replicated across partitions (verifier use)

    # per-core deg (sum of incoming edge vals), laid out [P, ntile]
    deg_w = np.bincount(edge_dst, weights=edge_vals.astype(np.float64),
                        minlength=N).astype(np.float32)
    degc = np.zeros((N_CORES, P, ntile), np.float32)
    for ci in range(N_CORES):
        d = np.zeros(ndst_pad, np.float32)
        d[:ndst] = deg_w[ci * ndst:(ci + 1) * ndst]
        degc[ci] = d.reshape(ntile, P).T
    pl.degc = degc
    # host-precomputed deg (x) bias, added at eviction: [P, ntile, OUT_F]
    pl.degb = np.ascontiguousarray(
        degc[:, :, :, None] * bias.astype(np.float32)[None, None, None, :])

    # --- edges sorted per core: (core, win, grp, tile, dl) ---
    src_a = edge_src.astype(np.int64)
    dst_a = edge_dst.astype(np.int64)
    val_a = edge_vals.astype(np.float32)

    core = dst_a // ndst
    dl = dst_a % ndst
    tile = dl // P
    grp = tile // TPG
    win = src_a // WINR
    order = np.lexsort((src_a, dl, tile, win, core))
    src_a, dst_a, val_a = src_a[order], dst_a[order], val_a[order]
    core, dl, tile, grp, win = (core[order], dl[order], tile[order],
                                grp[order], win[order])

    # run = consecutive edges with same (core, win, tile, dl)
    key_change = np.ones(len(src_a), bool)
    if len(src_a) > 1:
        key_change[1:] = ((core[1:] != core[:-1]) | (win[1:] != win[:-1]) |
                          (tile[1:] != tile[:-1]) | (dl[1:] != dl[:-1]))
    run_starts = np.nonzero(key_change)[0]
    run_lens = np.diff(np.append(run_starts, len(src_a)))
    r_core = core[run_starts]
    r_win = win[run_starts]
    r_grp = grp[run_starts]
    r_tile = tile[run_starts]
    r_dl = dl[run_starts]

    # --- continuous greedy chunk layout per (core, win, grp) bucket ---
    nbuck = NWIN * ngrp                    # buckets per core

    def bucket_id(win_, grp_):
        return win_ * ngrp + grp_

    def greedy(lens):
        # pieces: (run_index, take, chunk, slot_start, seg_slot)
        pieces = []
        c, s, d = 0, 0, 0
        for ri, ln in enumerate(lens):
            rem = ln
            while rem > 0:
                if s == P or d == MSEG:
                    c += 1
                    s, d = 0, 0
                take = min(P - s, rem)
                pieces.append((ri, take, c, s, d))
                s += take
                d += 1
                rem -= take
        return pieces, (c + 1 if (s > 0 or c == 0) else c)

    rb = (r_core * nbuck + bucket_id(r_win, r_grp)).astype(np.int64)
    rb_order = np.argsort(rb, kind="stable")
    chunks_cb = np.zeros((N_CORES, nbuck), np.int64)
    bucket_pieces = {}
    i = 0
    rb_sorted = rb[rb_order]
    while i < len(rb_sorted):
        j = i
        while j < len(rb_sorted) and rb_sorted[j] == rb_sorted[i]:
            j += 1
        ridx = rb_order[i:j]
        cb = int(rb_sorted[i])
        pieces, nch = greedy(run_lens[ridx])
        bucket_pieces[cb] = (ridx, pieces)
        chunks_cb[cb // nbuck, cb % nbuck] = nch
        i = j

    # static per-bucket chunk count: max over cores, padded to %4 so psum
    # groups of 4 chunks never straddle buckets
    chunks_real = chunks_cb.max(axis=0)            # used by >=1 core
    chunks_b = np.maximum(_ceil_to(chunks_real, 4), 4)

    # processing (and chunk layout) order: group-major, window-minor, so
    # each group's PSUM bank accumulates across all 4 windows
    chunk_off_b = np.zeros(nbuck, np.int64)
    off = 0
    for g in range(ngrp):
        for w in range(NWIN):
            b = bucket_id(w, g)
            chunk_off_b[b] = off
            off += int(chunks_b[b])
    CC = off                                 # total chunks per core
    TOT = CC * P                             # total edge slots per core

    # gather calls: one per bucket; sizes static
    gather_sizes = np.array([[int(chunks_b[bucket_id(w, g)]) * P
                              for g in range(ngrp)] for w in range(NWIN)],
                            np.int64)
    gather_off = np.zeros((NWIN, ngrp), np.int64)
    acc = 0
    for g in range(ngrp):
        for w in range(NWIN):
            gather_off[w, g] = acc
            acc += int(gather_sizes[w, g])
    assert acc == TOT

    # --- fill per-core slot arrays ---
    slot_src = np.full((N_CORES, TOT), -1, np.int16)    # idx within window
    Bf = np.zeros((N_CORES, P, CC * MSEG), np.float16)
    seg_chunk, seg_slot, seg_dl, seg_core = [], [], [], []
    for cb, (ridx, pieces) in bucket_pieces.items():
        ci, bid = cb // nbuck, cb % nbuck
        base_c = int(chunk_off_b[bid])
        pr = np.array([p[0] for p in pieces])
        pt = np.array([p[1] for p in pieces])
        pc = np.array([p[2] for p in pieces]) + base_c
        ps_ = np.array([p[3] for p in pieces])
        pd = np.array([p[4] for p in pieces])
        gri = ridx[pr]
        src_off = np.zeros(len(pieces), np.int64)
        for k in range(1, len(pieces)):
            if pr[k] == pr[k - 1]:
                src_off[k] = src_off[k - 1] + pt[k - 1]
        e_start = run_starts[gri] + src_off            # into sorted edges
        slot_start = pc * P + ps_                      # into slot arrays
        rep = np.repeat(np.arange(len(pieces)), pt)
        within = np.arange(len(rep)) - np.repeat(
            np.concatenate([[0], np.cumsum(pt)[:-1]]), pt)
        e_idx = e_start[rep] + within
        sl_idx = slot_start[rep] + within
        slot_src[ci, sl_idx] = (src_a[e_idx] -
                                win[e_idx] * WINR).astype(np.int16)
        Bf[ci, sl_idx % P, (sl_idx // P) * MSEG + pd[rep]] = (
            val_a[e_idx].astype(np.float16))
        seg_chunk.append(pc)
        seg_slot.append(pd)
        seg_dl.append(r_dl[gri])
        seg_core.append(np.full(len(pieces), ci))

    seg_chunk = np.concatenate(seg_chunk)
    seg_slot = np.concatenate(seg_slot)
    seg_dl = np.concatenate(seg_dl)
    seg_core = np.concatenate(seg_core)

    # --- static L2 program: mm per (psum-group j, tile), union over cores ---
    s_j = seg_chunk // 4
    s_tile = seg_dl // P
    jt = np.unique(s_j * ntile + s_tile)
    l2_mms = [(int(v) // ntile, int(v) % ntile) for v in jt]
    l2_mms.sort()
    NMM = len(l2_mms)
    mm_index = {jt_: i for i, jt_ in enumerate(l2_mms)}

    # chunk -> bucket, for start/stop flags per (bucket, tile)
    chunk_bucket = np.zeros(CC, np.int64)
    for w in range(NWIN):
        for g in range(ngrp):
            b = bucket_id(w, g)
            c0 = chunk_off_b[b]
            chunk_bucket[c0:c0 + chunks_b[b]] = b

    mm_start = np.zeros(NMM, bool)
    mm_stop = np.zeros(NMM, bool)
    seen = {}
    for i, (j, t) in enumerate(l2_mms):
        gg = int(chunk_bucket[4 * j]) % ngrp
        if (gg, t) not in seen:
            mm_start[i] = True
        seen[(gg, t)] = i
    for i in seen.values():
        mm_stop[i] = True

    # every tile of every group must receive at least one mm, else its
    # PSUM accumulator would be evicted uninitialized
    group_tiles = {}
    for (gg, t) in seen:
        group_tiles.setdefault(gg, set()).add(t)
    for g in range(ngrp):
        want = set(range(g * TPG, min((g + 1) * TPG, ntile)))
        assert group_tiles.get(g, set()) == want, (g, group_tiles.get(g))

    # S2 data
    s_row = (seg_chunk % 4) * MSEG + seg_slot
    s_mm = np.array([mm_index[(int(j), int(t))]
                     for j, t in zip(s_j, s_tile)])
    S2f = np.zeros((N_CORES, P, NMM * P), np.float16)
    S2f[seg_core, s_row, s_mm * P + (seg_dl % P)] = np.float16(1.0)

    # idx tensor: [16, TOT/16] wrap per gather call, replicated 8x
    IDX = np.zeros((N_CORES, 16, TOT // 16), np.int16)
    for w in range(NWIN):
        for g in range(ngrp):
            o, n = int(gather_off[w, g]), int(gather_sizes[w, g])
            IDX[:, :, o // 16:(o + n) // 16] = slot_src[
                :, o:o + n].reshape(N_CORES, n // 16, 16).transpose(0, 2, 1)
    # Per-bucket static gather count: the Q7 kernel truncates trailing
    # -1 indices, and the decode stage reserves DMA-ring space from
    # num_idxs_reg -- the two must agree on every core.  So we pad every
    # core's real slots with valid 0-indices up to the max fill across
    # cores (fill_max, passed as num_idxs_reg) and use -1 only beyond it.
    import os
    reg_counts = np.zeros((NWIN, ngrp), np.int64)
    for w in range(NWIN):
        for g in range(ngrp):
            o, n = int(gather_off[w, g]), int(gather_sizes[w, g])
            blk = slot_src[:, o:o + n]
            real = blk >= 0
            fills = np.where(real.any(axis=1),
                             n - np.argmax(real[:, ::-1], axis=1), 0)
            fmax = int(fills.max())
            reg_counts[w, g] = fmax
            for ci in range(N_CORES):
                blk[ci, fills[ci]:fmax] = 0
    if os.environ.get("K_NEGPAD", "1") == "0":
        slot_src[slot_src < 0] = 0
        reg_counts[:] = gather_sizes
    # (re)build IDX from the final slot_src
    IDX = np.zeros((N_CORES, 16, TOT // 16), np.int16)
    for w in range(NWIN):
        for g in range(ngrp):
            o, n = int(gather_off[w, g]), int(gather_sizes[w, g])
            IDX[:, :, o // 16:(o + n) // 16] = slot_src[
                :, o:o + n].reshape(N_CORES, n // 16, 16).transpose(0, 2, 1)
    pl.reg_counts = reg_counts
    IDX = np.tile(IDX, (1, 8, 1))          # -> [N_CORES, 128, TOT // 16]

    pl.chunks_b, pl.chunk_off_b = chunks_b, chunk_off_b
    pl.CC, pl.TOT, pl.NMM = CC, TOT, NMM
    pl.nbuck = nbuck
    pl.gather_sizes, pl.gather_off = gather_sizes, gather_off
    pl.l2_mms, pl.mm_start, pl.mm_stop = l2_mms, mm_start, mm_stop
    pl.group_tiles = group_tiles
    pl.IDX, pl.Bf, pl.S2f = IDX, Bf, S2f
    return pl


# ---------------------------------------------------------------------------
# Device program
# ---------------------------------------------------------------------------

def build_bass(pl):
    import os
    import concourse.bass as bass
    import concourse.mybir as mybir
    import concourse.tile as tile
    from concourse import bacc

    f16 = mybir.dt.float16
    f32 = mybir.dt.float32
    i16 = mybir.dt.int16

    NSWQ = int(os.environ.get("K_NSWQ", "4"))
    GMAX = int(os.environ.get("K_GMAX", "16384"))
    QROT = int(os.environ.get("K_QROT", "1"))
    SCALDMA = int(os.environ.get("K_SCALDMA", "1"))
    nc = bacc.Bacc("TRN2", target_bir_lowering=False, debug=False,
                   num_swdge_queues=NSWQ)

    OF = pl.OUT_F
    ngrp, ntile = pl.ngrp, pl.ntile
    dma2 = None  # set after nc engines exist

    xT_d = nc.dram_tensor("xt", [pl.kc, P, pl.hrows], f16,
                          kind="ExternalInput")
    W_d = nc.dram_tensor("w", [P, pl.kc, OF], f16, kind="ExternalInput")
    degb_d = nc.dram_tensor("degb", [P, ntile, OF], f32,
                            kind="ExternalInput")
    idx_d = nc.dram_tensor("idx", [P, pl.TOT // 16], i16,
                           kind="ExternalInput")
    B_d = nc.dram_tensor("bmat", [P, pl.CC * MSEG], f16, kind="ExternalInput")
    S2_d = nc.dram_tensor("s2", [P, pl.NMM * P], f16, kind="ExternalInput")
    out_d = nc.dram_tensor("out", [pl.ndst_pad, OF], f32,
                           kind="ExternalOutput")
    h_w = [nc.dram_tensor(f"hw{w}", [pl.win_rows[w], OF], f16)
           for w in range(NWIN)]

    RB = 512                       # projection row-batch
    max_n = int(pl.gather_sizes.max())

    with tile.TileContext(nc) as tc:
        with (
            tc.tile_pool(name="pconst", bufs=1) as pconst,
            tc.tile_pool(name="pxt", bufs=3) as pxt,
            tc.tile_pool(name="phs", bufs=3) as phs,
            tc.tile_pool(name="ppsum", bufs=2, space="PSUM") as ppsum,
            tc.tile_pool(name="pidx", bufs=2) as pidx,
            tc.tile_pool(name="pmsg", bufs=3) as pmsg,
            tc.tile_pool(name="pB", bufs=3) as pB,
            tc.tile_pool(name="pS2", bufs=3) as pS2,
            tc.tile_pool(name="pP", bufs=4) as pP,
            tc.tile_pool(name="pout", bufs=3) as pout,
            tc.tile_pool(name="psL1", bufs=2, space="PSUM") as psL1,
            tc.tile_pool(name="psL2", bufs=TPG, space="PSUM") as psL2,
        ):
            dma2 = nc.scalar if SCALDMA else nc.sync
            W_sb = pconst.tile([P, pl.kc, OF], f16)
            nc.sync.dma_start(W_sb[:], W_d[:])

            # warm the msg ring so pad slots never expose uninitialized
            # SBUF (0 * NaN would poison PSUM)
            for wi in range(3):
                mwarm = pmsg.tile([P, max_n // P, OF], f16, tag="msg",
                                  name=f"mwarm{wi}")
                nc.vector.memset(mwarm[:], 0.0)

            # ---------------- Phase 1: h = x @ W (per window) -------------
            for w in range(NWIN):
                r_base = w * WINR
                wr = pl.win_rows[w]
                n_rb = -(-wr // RB)
                for jb in range(n_rb):
                    r0 = jb * RB
                    nrows = min(RB, wr - r0)
                    nch = -(-nrows // P)
                    xt = pxt.tile([P, pl.kc, RB], f16, tag="xt")
                    nc.sync.dma_start(
                        xt[:, :, :nrows],
                        xT_d[:, :, r_base + r0:r_base + r0 + nrows].rearrange(
                            "k p c -> p k c"),
                    )
                    ps = ppsum.tile([P, RB], f32, tag="pj")
                    for rc in range(nch):
                        for k in range(pl.kc):
                            nc.tensor.matmul(
                                ps[:, rc * P:(rc + 1) * P],
                                lhsT=xt[:, k, rc * P:(rc + 1) * P],
                                rhs=W_sb[:, k, :],
                                start=(k == 0),
                                stop=(k == pl.kc - 1),
                            )
                    hs = phs.tile([P, RB], f16, tag="hs")
                    nc.vector.tensor_copy(hs[:, :nrows], ps[:, :nrows])
                    nc.sync.dma_start(
                        h_w[w][r0:r0 + nrows, :].rearrange(
                            "(c p) f -> p c f", p=P),
                        hs[:, :nrows].rearrange("p (c f) -> p c f", f=OF),
                    )

            # ---------------- Phase 2: gather + L1 + L2 ----------------
            mm_i = 0          # global L2 mm counter
            call_i = 0
            for g in range(ngrp):
                tiles_g = sorted(pl.group_tiles[g])
                # one PSUM bank per dst-tile accumulator: concurrently open
                # matmul accumulation groups must not share a bank (start=
                # True clears has_written at bank granularity)
                l2ps = {t: psL2.tile([P, OF], f32, tag="l2",
                                     name=f"l2ps_{g}_{t}")
                        for t in tiles_g}
                for w in range(NWIN):
                    bkt = w * ngrp + g
                    n = int(pl.gather_sizes[w, g])
                    o = int(pl.gather_off[w, g])
                    c_lo = int(pl.chunk_off_b[bkt])
                    nch_b = int(pl.chunks_b[bkt])

                    # static num_idxs = max fill across cores: the Q7
                    # idx-conversion loop, desc-gen and the decode-side ring
                    # accounting all run over exactly this count; slots
                    # beyond it keep (warmed) stale data and zero B columns
                    nreg = int(pl.reg_counts[w, g])
                    n16 = _ceil_to(max(nreg, 16), 16)
                    ixt = pidx.tile([P, n // 16], i16, tag="idx")
                    dma2.dma_start(ixt[:, :n16 // 16],
                                   idx_d[:, o // 16:(o + n16) // 16])
                    mt = pmsg.tile([P, n // P, OF], f16, tag="msg")
                    for q0 in range(0, nreg, GMAX):
                        qn = min(GMAX, nreg - q0)
                        nc.gpsimd.dma_gather(
                            out_ap=mt[:, q0 // P:_ceil_to(q0 + qn, P) // P, :],
                            in_ap=h_w[w][:, :],
                            idxs_ap=ixt[:, q0 // 16:_ceil_to(q0 + qn, 16) // 16],
                            num_idxs=qn,
                            num_idxs_reg=qn,
                            elem_size=OF,
                            single_packet=False,
                            queue_num=(call_i % NSWQ) if QROT else 0,
                        )
                        call_i += 1

                    # psum-groups of 4 chunks; B/S2 tiles cover 16 chunks
                    Bt, s2t, s2_mm0 = None, None, 0
                    for pg in range(nch_b // 4):
                        c0 = c_lo + pg * 4
                        if pg % 4 == 0:
                            bch = min(16, nch_b - pg * 4)
                            Bt = pB.tile([P, 16 * MSEG], f16, tag="B")
                            dma2.dma_start(
                                Bt[:, :bch * MSEG],
                                B_d[:, c0 * MSEG:(c0 + bch) * MSEG])
                            # all l2 mms whose psum-group is in this block
                            j_hi = c0 // 4 + bch // 4
                            s2_mm0 = mm_i
                            mm_hi = mm_i
                            while (mm_hi < pl.NMM and
                                   pl.l2_mms[mm_hi][0] < j_hi):
                                mm_hi += 1
                            if mm_hi > mm_i:
                                s2t = pS2.tile(
                                    [P, (mm_hi - s2_mm0) * P], f16,
                                    tag="s2", name=f"s2_{g}_{w}_{pg}")
                                dma2.dma_start(
                                    s2t[:],
                                    S2_d[:, s2_mm0 * P:mm_hi * P])
                        ps1 = psL1.tile([P, P], f32, tag="ps1")
                        for cc in range(4):
                            c = c0 + cc
                            boff = ((pg % 4) * 4 + cc) * MSEG
                            nc.tensor.matmul(
                                ps1[32 * cc:32 * (cc + 1), :],
                                lhsT=Bt[:, boff:boff + MSEG],
                                rhs=mt[:, c - c_lo, :],
                                start=True,
                                stop=True,
                                tile_position=(0, 32 * cc),
                            )
                        Pt = pP.tile([P, OF], f16, tag="P")
                        nc.vector.tensor_copy(Pt[:], ps1[:])
                        # L2 mms for this psum-group (j == global pg index)
                        j = c0 // 4
                        while mm_i < pl.NMM and pl.l2_mms[mm_i][0] == j:
                            _, t = pl.l2_mms[mm_i]
                            q = mm_i - s2_mm0
                            nc.tensor.matmul(
                                l2ps[t],
                                lhsT=s2t[:, q * P:(q + 1) * P],
                                rhs=Pt[:],
                                start=bool(pl.mm_start[mm_i]),
                                stop=bool(pl.mm_stop[mm_i]),
                            )
                            mm_i += 1

                # evict group's dst tiles: out = l2 + deg (x) bias
                dbt = pout.tile([P, TPG, OF], f32, tag="degb")
                t0 = tiles_g[0]
                nc.sync.dma_start(dbt[:, :len(tiles_g), :],
                                  degb_d[:, t0:t0 + len(tiles_g), :])
                for q, t in enumerate(tiles_g):
                    ot = pout.tile([P, OF], f32, tag="out")
                    nc.vector.tensor_add(ot[:], dbt[:, q, :], l2ps[t])
                    nc.sync.dma_start(out_d[t * P:(t + 1) * P, :], ot[:])

    nc.compile()
    return nc


# ---------------------------------------------------------------------------
# Entry point
# ---------------------------------------------------------------------------

def kernel(x, edge_src, edge_dst, edge_vals, weight, bias,
           _want_trace=False, _n_cores=None):
    x = np.asarray(x)
    edge_src = np.asarray(edge_src)
    edge_dst = np.asarray(edge_dst)
    edge_vals = np.asarray(edge_vals)
    weight = np.asarray(weight)
    bias = np.asarray(bias)

    pl = build_plan(x, edge_src, edge_dst, edge_vals, weight, bias)
    nc = build_bass(pl)

    from concourse.bass_utils import run_bass_kernel_spmd

    ncores = N_CORES if _n_cores is None else _n_cores
    in_maps = []
    for ci in range(ncores):
        in_maps.append({
            "xt": pl.xT,
            "w": pl.W,
            "degb": np.ascontiguousarray(pl.degb[ci]),
            "idx": np.ascontiguousarray(pl.IDX[ci]),
            "bmat": np.ascontiguousarray(pl.Bf[ci]),
            "s2": np.ascontiguousarray(pl.S2f[ci]),
        })
    res = run_bass_kernel_spmd(nc, in_maps, core_ids=list(range(ncores)),
                               trace=_want_trace)
    outs = [res.results[ci]["out"][:pl.ndst, :] for ci in range(ncores)]
    if ncores < N_CORES:
        outs += [np.zeros((pl.ndst, pl.OUT_F), np.float32)] * (N_CORES - ncores)
    full = np.concatenate(outs, axis=0).astype(np.float32)
    if _want_trace:
        kernel._last_results = res
    return full
